# revision 1
# baseline (speedup 1.0000x reference)
"""DeepFM inference kernel for Trainium2 (8 NeuronCores, batch-parallel).

Model (from the reference):
  - per-feature LayerNorm over a length-1 axis degenerates to ln_beta
    (normalized value is exactly 0), so dense_x never affects the output;
    beta folds into the MLP / linear biases on the host.
  - embedding gather tables[f, ids[b,f]] -> emb [B, 26, 64]
  - linear   = emb_flat @ w_lin[13:] + (b_lin + beta @ w_lin[:13])
  - cross    = 0.5 * (sum_d S_d^2 - Q),  S = sum_f emb, Q = sum_{f,d} emb^2
  - h1 = relu(emb_flat @ w1[13:] + b1 + beta @ w1[:13]); h2 = relu(h1 @ w2 + b2)
  - out = sigmoid((linear + cross) * sum(w_out) + h2 @ w_out + b_out)

Sharding: batch-parallel across the 8 cores (2048 rows each), embedding
tables replicated in each core's DRAM.  No collectives needed.

Per-core device pipeline (16 batch subtiles of 128 rows):
  - gpsimd indirect DMA gathers 26 x 256B table rows per batch row into
    an SBUF tile [128b, 1664e] (offsets precomputed on host as flat
    row indices into the [26*100000, 64] table).
  - ACT squares each gathered tile with accum_out -> Q per batch row.
  - PE transposes [128b,128e] blocks, DVE copies PSUM->SBUF casting to
    fp32r, then PE accumulates h1^T k-tile matmuls plus an aux matmul
    whose stationary is [stacked 64-identities | w_lin], giving S^T and
    the linear term in one extra pass of the same moving data.
  - finals: relu/square on ACT, small PE matmuls fold everything into a
    [1, 512] pre-sigmoid row (including -0.5*Sw*Q via a negated scaled
    identity), sigmoid on ACT, one DMA out.

Trainium ISA allows ONE semaphore wait per instruction, and Tile does
not split excess waits, so the program is arranged to give every
instruction at most one *fresh* cross-engine dependency:
  - all constants arrive in a single packed DMA (one semaphore);
  - per-engine one-time warmup ops observe that semaphore first;
  - tiles read by a different engine than their writer either live in
    never-reused pools (q_ch, arg1, tokens) or are released in a way
    already covered by each consumer's monotone engine-clock;
  - a tiny gpsimd copy of q_ch each chunk lets the Pool engine observe
    ACT progress before emb slots are reused.
"""

import sys

sys.path.insert(0, "/opt/trn_rl_repo")

import numpy as np

from concourse import bacc, bass, mybir, tile
from concourse.bass_utils import run_bass_kernel_spmd

F32 = mybir.dt.float32
F32R = mybir.dt.float32r if __import__("os").environ.get("USE_F32R","1")=="1" else mybir.dt.float32
I32 = mybir.dt.int32
BF16 = mybir.dt.bfloat16
AF = mybir.ActivationFunctionType
OP = mybir.AluOpType

B = 16384
N_DENSE = 13
F = 26
V = 100000
D = 64
H1 = 128
H2 = 64
NCORES = 8
BL = B // NCORES            # 2048 rows per core
P = 128                     # partitions
KT = (F * D) // P           # 13 k-tiles over the 1664 embedding dims
NSUB = BL // P              # 16 subtiles of 128 rows
SPC = 4                     # subtiles per chunk
NCHUNK = NSUB // SPC        # 4 chunks of 512 rows
CW = SPC * P                # chunk width (512)

# packed-constant column offsets (in 4-byte units)
C_OFFS = 0
C_W1 = C_OFFS + NSUB * F          # 416
C_AUX = C_W1 + KT * H1            # 416+1664
C_W2 = C_AUX + KT * 65
C_WC = C_W2 + H2
C_SI = C_WC + 1
C_ID = C_SI + P
C_B1 = C_ID + P
C_B2 = C_B1 + 1
C_SC = C_B2 + 1
C_Z = C_SC + 2
C_IDB = C_Z + 1                   # bf16 identity, 64 f32 cols
NW = C_IDB + 64

_PROGRAM = None


def _build_program():
    nc = bacc.Bacc(None)

    tables_d = nc.dram_tensor("tables", [F * V, D], BF16, kind="ExternalInput")
    pack_d = nc.dram_tensor("wpack", [P, NW], F32R, kind="ExternalInput")
    offs_d = nc.dram_tensor("offs", [P, NSUB * F], I32, kind="ExternalInput")
    out_d = nc.dram_tensor("out", [1, BL], F32, kind="ExternalOutput")
    DBG = __import__("os").environ.get("KDBG", "0") == "1"
    if DBG:
        dbg_d = nc.dram_tensor("dbg", [1, 6 * BL], F32, kind="ExternalOutput")
        dbge_d = nc.dram_tensor("dbge", [P, F * D], F32, kind="ExternalOutput")
        dbgt_d = nc.dram_tensor("dbgt", [P, CW], F32, kind="ExternalOutput")

    with tile.TileContext(nc) as tc:
        with (
            tc.tile_pool(name="const", bufs=1) as cpool,
            tc.tile_pool(name="emb", bufs=12) as embpool,
            tc.tile_pool(name="sq", bufs=2) as sqpool,
            tc.tile_pool(name="embT", bufs=3) as tpool,
            tc.tile_pool(name="work", bufs=2) as wpool,
            tc.tile_pool(name="once", bufs=4) as opool,
            tc.tile_pool(name="ps_tr", bufs=2, space="PSUM") as ps_tr,
            tc.tile_pool(name="ps_h1", bufs=2, space="PSUM") as ps_h1,
            tc.tile_pool(name="ps_aux", bufs=1, space="PSUM") as ps_aux,
            tc.tile_pool(name="ps_warm", bufs=1, space="PSUM") as ps_warm,
            tc.tile_pool(name="ps_sm", bufs=2, space="PSUM") as ps_sm,
        ):
            pack = cpool.tile([P, NW], F32R)
            nc.sync.dma_start(out=pack[:], in_=pack_d[:])
            offs_sb = cpool.tile([P, NSUB * F], I32)
            nc.sync.dma_start(out=offs_sb[:], in_=offs_d[:])

            packf = pack[:].bitcast(F32)
            w1_sb = pack[:, C_W1:C_AUX]
            aux_sb = pack[:, C_AUX:C_W2]
            w2_sb = pack[:, C_W2:C_WC]
            wc_sb = pack[:, C_WC:C_SI]
            si_sb = packf[:, C_SI:C_ID]
            id_sb = packf[:, C_ID:C_B1]
            b1_sb = packf[:, C_B1:C_B2]
            b2_sb = packf[:, C_B2:C_SC]      # rows 0..63 hold b2
            sc_sb = packf[:, C_SC:C_Z]       # [0,0]=Sw  [0,1]=sigmoid bias
            zcol = packf[:, C_Z:C_IDB]       # zeros (AP bias for Square)
            idb_sb = packf[:, C_IDB:NW].bitcast(BF16)   # [P, 128] bf16 identity

            # per-engine warmups: one op each so every engine's clock
            # observes the packed-constant DMA semaphore exactly once
            wtile = ps_warm.tile([P, P], F32, tag="warm")
            nc.tensor.transpose(out=wtile[:], in_=id_sb, identity=id_sb)
            actw = opool.tile([1, 1], F32, tag="actw")
            nc.scalar.activation(out=actw[:], in_=sc_sb[0:1, 0:1], func=AF.Copy)
            dvew = opool.tile([1, 2], F32, tag="dvew")
            nc.vector.tensor_copy(out=dvew[:], in_=sc_sb[0:1, :])

            outsb = cpool.tile([1, BL], F32)
            sq = cpool.tile([P, F * D], F32)   # single ACT scratch, WAW only
            if DBG:
                dbgsb = cpool.tile([1, 6 * BL], F32)

            prev_q = None
            for c in range(NCHUNK):
                # Pool observes ACT's chunk-(c-1) progress here, a full chunk
                # after those squares issued — no stall — while still fencing
                # emb-slot reuse (slots recycle at c+3 with bufs=12).
                if prev_q is not None:
                    tok = opool.tile([P, 1], F32, tag="tok")
                    nc.gpsimd.tensor_copy(out=tok[:], in_=prev_q[:, SPC - 1 : SPC])
                q_ch = opool.tile([P, SPC], F32, tag="qch")
                embs = []
                for s in range(SPC):
                    t = c * SPC + s
                    emb = embpool.tile([P, F * D], BF16, tag="emb")
                    # HW semantics: one index per partition, contiguous run
                    # per index — so one instruction per (subtile, field)
                    for f in range(F):
                        nc.gpsimd.indirect_dma_start(
                            out=emb[:, f * D : (f + 1) * D],
                            out_offset=None,
                            in_=tables_d[:],
                            in_offset=bass.IndirectOffsetOnAxis(
                                ap=offs_sb[:, t * F + f : t * F + f + 1], axis=0
                            ),
                        )
                    embs.append(emb)
                    nc.scalar.activation(
                        out=sq[:], in_=emb[:], func=AF.Square,
                        bias=zcol[:, 0:1],
                        accum_out=q_ch[:, s : s + 1],
                    )
                prev_q = q_ch

                psum_h1 = ps_h1.tile([P, CW], F32, tag="h1")
                psum_aux = ps_aux.tile([65, CW], F32, tag="aux")
                for kt in range(KT):
                    ptr = ps_tr.tile([P, CW], BF16, tag="tr")
                    for s in range(SPC):
                        nc.tensor.transpose(
                            out=ptr[:, s * P : (s + 1) * P],
                            in_=embs[s][:, kt * P : (kt + 1) * P],
                            identity=idb_sb,
                        )
                    embT = tpool.tile([P, CW], F32R, tag="embT")
                    nc.vector.tensor_copy(out=embT[:], in_=ptr[:])
                    if DBG and c == 0 and kt == 0:
                        nc.sync.dma_start(out=dbge_d[:], in_=embs[0][:])
                        nc.sync.dma_start(out=dbgt_d[:], in_=embT[:].bitcast(F32))
                    nc.tensor.matmul(
                        out=psum_h1[:],
                        lhsT=w1_sb[:, kt * H1 : (kt + 1) * H1],
                        rhs=embT[:],
                        start=(kt == 0), stop=(kt == KT - 1),
                    )
                    nc.tensor.matmul(
                        out=psum_aux[:],
                        lhsT=aux_sb[:, kt * 65 : (kt + 1) * 65],
                        rhs=embT[:],
                        start=(kt == 0), stop=(kt == KT - 1),
                    )

                h1r = wpool.tile([P, CW], F32R, tag="h1r")
                nc.scalar.activation(
                    out=h1r[:], in_=psum_h1[:], func=AF.Relu,
                    bias=b1_sb[:, 0:1], scale=1.0,
                )
                psum_h2 = ps_sm.tile([H2, CW], F32, tag="sm")
                nc.tensor.matmul(
                    out=psum_h2[:], lhsT=w2_sb[:],
                    rhs=h1r[:], start=True, stop=True,
                )
                s2h2 = wpool.tile([P, CW], F32R, tag="s2h2")
                nc.scalar.activation(
                    out=s2h2[0:H2, :], in_=psum_aux[0:H2, :], func=AF.Square,
                    bias=zcol[0:H2, 0:1],
                )
                nc.scalar.activation(
                    out=s2h2[H2:P, :], in_=psum_h2[:], func=AF.Relu,
                    bias=b2_sb[0:H2, 0:1], scale=1.0,
                )
                psum_fin = ps_sm.tile([1, CW], F32, tag="sm")
                nc.tensor.matmul(
                    out=psum_fin[:], lhsT=wc_sb[:],
                    rhs=s2h2[:], start=True, stop=False,
                )
                # accumulate -0.5*Sw*Q via q_ch columns against the negated
                # scaled identity, one 128-wide slice per subtile
                for s in range(SPC):
                    nc.tensor.matmul(
                        out=psum_fin[0:1, s * P : (s + 1) * P],
                        lhsT=q_ch[:, s : s + 1], rhs=si_sb,
                        start=False, stop=(s == SPC - 1),
                        skip_group_check=True,
                    )
                arg1 = opool.tile([1, CW], F32, tag="arg1")
                nc.vector.tensor_scalar(
                    out=arg1[:], in0=psum_aux[64:65, :],
                    scalar1=sc_sb[0:1, 0:1], scalar2=None, op0=OP.mult,
                )
                nc.vector.tensor_tensor(
                    out=arg1[:], in0=arg1[:], in1=psum_fin[:], op=OP.add,
                )
                nc.scalar.activation(
                    out=outsb[0:1, c * CW : (c + 1) * CW], in_=arg1[:],
                    func=AF.Sigmoid, bias=sc_sb[0:1, 1:2], scale=1.0,
                )
                if DBG:
                    for di, ap in enumerate([
                        psum_aux[64:65, :], psum_fin[0:1, :], arg1[0:1, :],
                        s2h2[0:1, :].bitcast(F32), s2h2[H2:H2+1, :].bitcast(F32),
                        h1r[0:1, :].bitcast(F32),
                    ]):
                        nc.vector.tensor_copy(
                            out=dbgsb[0:1, di * BL + c * CW : di * BL + (c + 1) * CW],
                            in_=ap,
                        )

            nc.sync.dma_start(out=out_d[:], in_=outsb[:])
            if DBG:
                nc.sync.dma_start(out=dbg_d[:], in_=dbgsb[:])

    nc.compile()
    return nc


def _get_program():
    global _PROGRAM
    if _PROGRAM is None:
        _PROGRAM = _build_program()
    return _PROGRAM


def _host_prep(dense_x, sparse_ids, tables, ln_gamma, ln_beta,
               w_lin, b_lin, w1, b1, w2, b2, w_out, b_out):
    del dense_x, ln_gamma  # output is mathematically independent of both

    import ml_dtypes
    tables = np.ascontiguousarray(
        np.asarray(tables, dtype=np.float32).reshape(F * V, D).astype(ml_dtypes.bfloat16)
    )
    ids = np.asarray(sparse_ids)
    rows = (np.arange(F, dtype=np.int64)[None, :] * V + ids).astype(np.int32)  # [B, F]

    ln_beta = np.asarray(ln_beta, dtype=np.float64)
    w1 = np.asarray(w1, dtype=np.float64)
    w_lin = np.asarray(w_lin, dtype=np.float64)
    w_out_f = np.asarray(w_out, dtype=np.float64).reshape(H2)
    sw = float(w_out_f.sum())

    b1_eff = (np.asarray(b1, dtype=np.float64) + ln_beta @ w1[:N_DENSE]).astype(np.float32)
    blin_eff = float(np.asarray(b_lin, dtype=np.float64)[0] + ln_beta @ w_lin[:N_DENSE, 0])
    w1e = np.asarray(w1[N_DENSE:], dtype=np.float32)          # [1664, 128]
    wlin_e = np.asarray(w_lin[N_DENSE:, 0], dtype=np.float32)  # [1664]

    pack = np.zeros((P, NW), dtype=np.float32)
    # w1 packed: [p, kt*H1 + j] = w1e[kt*128 + p, j]
    pack[:, C_W1:C_AUX] = np.ascontiguousarray(
        w1e.reshape(KT, P, H1).transpose(1, 0, 2)
    ).reshape(P, KT * H1)
    # aux stationary per k-tile: cols 0..63 stacked identities, col 64 w_lin
    aux = np.zeros((KT, P, 65), dtype=np.float32)
    r = np.arange(P)
    aux[:, r, r % D] = 1.0
    aux[:, :, D] = wlin_e.reshape(KT, P)
    pack[:, C_AUX:C_W2] = np.ascontiguousarray(aux.transpose(1, 0, 2)).reshape(P, KT * 65)
    pack[:, C_W2:C_WC] = np.asarray(w2, dtype=np.float32)
    pack[:H2, C_WC] = 0.5 * sw
    pack[H2:, C_WC] = w_out_f.astype(np.float32)
    pack[:, C_SI:C_ID] = (-0.5 * sw) * np.eye(P, dtype=np.float32)
    pack[:, C_ID:C_B1] = np.eye(P, dtype=np.float32)
    pack[:, C_B1] = b1_eff
    pack[:H2, C_B2] = np.asarray(b2, dtype=np.float32)
    sig_bias = float(np.asarray(b_out, dtype=np.float64)[0] + sw * blin_eff)
    pack[0, C_SC] = sw
    pack[0, C_SC + 1] = sig_bias
    # pack[:, C_Z] stays zero
    idb = np.eye(P, dtype=ml_dtypes.bfloat16)
    pack[:, C_IDB:NW] = np.ascontiguousarray(idb).view(np.uint16).astype(np.uint32).reshape(P, P // 2, 2)[..., 0].astype(np.float32) * 0  # placeholder, set below
    pack[:, C_IDB:NW] = np.ascontiguousarray(idb).view(np.float32)

    in_maps = []
    for core in range(NCORES):
        rc = rows[core * BL : (core + 1) * BL]  # [2048, 26]
        offs = np.ascontiguousarray(
            rc.reshape(NSUB, P, F).transpose(1, 0, 2)
        ).reshape(P, NSUB * F)
        in_maps.append({"tables": tables, "wpack": pack, "offs": offs})
    return in_maps


def kernel(**inputs):
    nc = _get_program()
    in_maps = _host_prep(**inputs)
    res = run_bass_kernel_spmd(nc, in_maps, list(range(NCORES)))
    out = np.concatenate([res.results[c]["out"].reshape(BL) for c in range(NCORES)])
    return out.reshape(B, 1).astype(np.float32)

